# revision 16
# baseline (speedup 1.0000x reference)
"""SE/attention block (avgpool31s16 -> 1x1 conv relu -> 1x1 conv sigmoid -> upsample*x)
on 8 TRN2 NeuronCores, batch-parallel (core b owns x[b]).

out[b,c,h,w] = x[b,c,h,w] * sigmoid(w2 @ relu(w1 @ p[b,:,h//16,w//16] + b1) + b2)[c]
where p = AvgPool2d(k=31, s=16, pad=15, count_include_pad=False)(x).

Key identity: pooling is linear and per-channel, so w1 @ Pool(x) = Pool(w1 @ x).
The TensorEngine does the 128->32 channel contraction on the raw stream (fp32r,
single-pass); the pooled sums are then computed on the 32-channel result y with
the VectorEngine using separable 16-block sums:
  window_j = block_{j-1} + block_j - col(16(j-1))   (j>=1), window_0 = block_0
(31-wide stride-16 windows; only the first window is clipped: counts 16 vs 31).

v2 layout: per 16-row tile, in-DMA on sync (HWDGE); the final x*s multiply is
two fused broadcast tensor_tensor ops (rows 0:8 on Vector, rows 8:16 on GpSimd)
against a stride-0 view of s; out-DMA half0 on scalar (HWDGE), half1 on gpsimd
(SWDGE).  The s-chain for tile i-1 is issued after tile i's y-matmuls so the PE
queue never stalls on the Vector chain.  All consts ride one packed DMA first.
"""

import numpy as np
from contextlib import ExitStack

import concourse.bass as bass
import concourse.tile as tile
from concourse import bacc, mybir
from concourse.bass_utils import run_bass_kernel_spmd

F32 = mybir.dt.float32
F32R = mybir.dt.float32r
AF = mybir.ActivationFunctionType

C, CR = 128, 32          # channels in / squeezed
H, W = 256, 256
NT = 16                  # h-tiles of 16 rows
TH = 16                  # rows per tile
NJ = 16                  # pooled cols
G0, G1 = 1.0 / 16.0, 1.0 / 31.0  # 1/count for edge/interior windows
NCOL = 210               # packed const columns


def _se_body(ctx, tc, out, x, cpack):
    nc = tc.nc

    consts = ctx.enter_context(tc.tile_pool(name="consts", bufs=1))
    xpool = ctx.enter_context(tc.tile_pool(name="xpool", bufs=4))
    opool = ctx.enter_context(tc.tile_pool(name="opool", bufs=5))
    small = ctx.enter_context(tc.tile_pool(name="small", bufs=3))
    ypsum = ctx.enter_context(tc.tile_pool(name="ypsum", bufs=3, space="PSUM"))
    spsum = ctx.enter_context(tc.tile_pool(name="spsum", bufs=2, space="PSUM"))

    # one packed const DMA on the sync queue ahead of the x stream
    cp = consts.tile([C, NCOL], F32)
    nc.sync.dma_start(out=cp, in_=cpack)
    w1t_s = cp[:, 0:32]            # [128, 32] lhsT of the 128->32 contraction
    qmat_s = cp[:, 32:64]          # [128, 32] 4-stacked I_32 (sums row groups)
    b2_s = cp[:, 64:65]            # [128, 1]
    b1_s = cp[0:CR, 65:66]         # [32, 1]
    w2t_s = cp[0:CR, 66:194]       # [32, 128] lhsT of the 32->128 expansion
    nj_s = cp[0:CR, 194:210]       # [32, 16] per-j 1/count_w factors

    def load(i):
        xt = xpool.tile([C, TH, NJ, 16], F32)
        if i == 0:
            # split the pipeline-fill tile so its first matmuls start ~4x
            # sooner (region-level deps let each row-quarter gate separately)
            for s in range(4):
                nc.sync.dma_start(out=xt[:, 4 * s : 4 * s + 4], in_=x[:, 4 * s : 4 * s + 4])
        else:
            nc.sync.dma_start(out=xt, in_=x[:, TH * i : TH * (i + 1)])
        # y[32q+o, hl, j, wi] = sum_c w1[o,c] * x[c, 4q+hl, 16j+wi]  (fp32r)
        y = ypsum.tile([C, 4, NJ, 16], F32)
        for q in range(4):
            for a in range(2):
                r = 4 * q + 2 * a
                nc.tensor.matmul(
                    out=y[32 * q : 32 * q + 32, 2 * a : 2 * a + 2, :, :],
                    lhsT=w1t_s,
                    rhs=xt[:, r : r + 2],
                    start=True,
                    stop=True,
                    tile_position=(0, 32 * q),
                )
        return xt, y

    tail = {"prev": None}

    def chain(i, xt, y):
        # 16-wide block sums along w, then stride-16 window sums (kernel 31)
        bs = small.tile([C, 4, NJ], F32)
        nc.vector.reduce_sum(out=bs, in_=y, axis=mybir.AxisListType.X)
        ws = small.tile([C, 4, NJ], F32)
        nc.vector.tensor_copy(out=ws[:, :, 0:1], in_=bs[:, :, 0:1])
        nc.vector.tensor_add(out=ws[:, :, 1:NJ], in0=bs[:, :, 0 : NJ - 1], in1=bs[:, :, 1:NJ])
        nc.vector.tensor_sub(out=ws[:, :, 1:NJ], in0=ws[:, :, 1:NJ], in1=y[:, :, 0 : NJ - 1, 0])

        # sum the 4 local rows per partition group, then the 4 groups via PE
        cs = small.tile([C, NJ], F32)
        nc.vector.reduce_sum(out=cs, in_=ws.rearrange("p a b -> p b a"), axis=mybir.AxisListType.X)
        full_ps = spsum.tile([CR, NJ], F32, tag="sp")
        nc.tensor.matmul(out=full_ps, lhsT=qmat_s, rhs=cs, start=True, stop=True)

        # window rows i: last 15 rows of tile i-1 + all 16 of tile i
        p_un = small.tile([CR, NJ], F32)
        if i == 0:
            nc.vector.tensor_copy(out=p_un, in_=full_ps)
        else:
            nc.vector.tensor_add(out=p_un, in0=full_ps, in1=tail["prev"])
        if i < NT - 1:
            tail_cur = small.tile([CR, NJ], F32)
            nc.vector.tensor_sub(out=tail_cur, in0=full_ps, in1=ws[0:CR, 0, :])
            tail["prev"] = tail_cur

        # h = relu(g_i * (p_un * g_j) + b1);  s = sigmoid(w2 @ h + b2)
        tmp = small.tile([CR, NJ], F32)
        nc.vector.tensor_mul(out=tmp, in0=p_un, in1=nj_s)
        h_s = small.tile([CR, NJ], F32)
        nc.scalar.activation(
            out=h_s, in_=tmp, func=AF.Relu, bias=b1_s, scale=(G0 if i == 0 else G1)
        )
        s_ps = spsum.tile([C, NJ], F32, tag="sp")
        nc.tensor.matmul(out=s_ps, lhsT=w2t_s, rhs=h_s, start=True, stop=True)
        s_s = small.tile([C, NJ], F32)
        nc.scalar.activation(out=s_s, in_=s_ps, func=AF.Sigmoid, bias=b2_s, scale=1.0)

        # out tile = x tile * s: materialize srow so the muls' innermost dim
        # has a real stride (a stride-0 inner dim runs ~3x slower on DVE),
        # then one fused multiply per row-half (Vector 0:8, GpSimd 8:16);
        # srow is built on the Scalar engine right behind the sigmoid
        srow = small.tile([C, NJ, 16], F32)
        nc.vector.tensor_copy(out=srow, in_=s_s.unsqueeze(2).broadcast_to([C, NJ, 16]))
        ot = opool.tile([C, TH, NJ, 16], F32)
        s_b = srow.unsqueeze(1).broadcast_to([C, 8, NJ, 16])
        nc.vector.tensor_mul(out=ot[:, 0:8], in0=xt[:, 0:8], in1=s_b)
        nc.gpsimd.tensor_mul(out=ot[:, 8:16], in0=xt[:, 8:16], in1=s_b)
        nc.scalar.dma_start(out=out[:, TH * i : TH * i + 8], in_=ot[:, 0:8])
        nc.gpsimd.dma_start(out=out[:, TH * i + 8 : TH * (i + 1)], in_=ot[:, 8:16])

    # tile 0's chain is issued immediately (PE stalls are free during the
    # pipeline fill and this starts the out stream ~5us earlier); from tile 1
    # on, chain(i-1) is issued after tile i's y-matmuls so the PE queue never
    # stalls on the Vector chain in steady state
    first = load(0)
    chain(0, *first)
    prev = load(1)
    for i in range(2, NT + 1):
        cur = load(i) if i < NT else None
        chain(i - 1, *prev)
        prev = cur


def build_nc():
    nc = bacc.Bacc("TRN2", target_bir_lowering=False, debug=False)
    x = nc.dram_tensor("x", [C, H, NJ, 16], F32, kind="ExternalInput").ap()
    cpack = nc.dram_tensor("cpack", [C, NCOL], F32, kind="ExternalInput").ap()
    out = nc.dram_tensor("out", [C, H, NJ, 16], F32, kind="ExternalOutput").ap()
    with tile.TileContext(nc) as tc:
        with ExitStack() as ctx:
            _se_body(ctx, tc, out, x, cpack)
    nc.compile()
    return nc


def make_in_maps(x, w1, b1, w2, b2):
    cpack = np.zeros((C, NCOL), dtype=np.float32)
    cpack[:, 0:32] = w1.T                                    # [128, 32]
    cpack[:, 32:64] = np.tile(np.eye(CR, dtype=np.float32), (4, 1))
    cpack[:, 64] = b2
    cpack[0:CR, 65] = b1
    cpack[0:CR, 66:194] = w2.T                               # [32, 128]
    gj = np.full(NJ, G1, dtype=np.float32)
    gj[0] = G0
    cpack[0:CR, 194:210] = gj[None, :]
    return [
        {
            "x": np.ascontiguousarray(x[b]).reshape(C, H, NJ, 16),
            "cpack": cpack,
        }
        for b in range(x.shape[0])
    ]


_NC_CACHE = {}


def _get_nc():
    if "nc" not in _NC_CACHE:
        _NC_CACHE["nc"] = build_nc()
    return _NC_CACHE["nc"]


def kernel(x, w1, b1, w2, b2):
    nc = _get_nc()
    in_maps = make_in_maps(x, w1, b1, w2, b2)
    res = run_bass_kernel_spmd(nc, in_maps, core_ids=list(range(8)))
    return np.stack(
        [res.results[i]["out"].reshape(C, H, W) for i in range(8)], axis=0
    )


# revision 17
# speedup vs baseline: 1.0967x; 1.0967x over previous
"""SE/attention block (avgpool31s16 -> 1x1 conv relu -> 1x1 conv sigmoid -> upsample*x)
on 8 TRN2 NeuronCores, batch-parallel (core b owns x[b]).

out[b,c,h,w] = x[b,c,h,w] * sigmoid(w2 @ relu(w1 @ p[b,:,h//16,w//16] + b1) + b2)[c]
where p = AvgPool2d(k=31, s=16, pad=15, count_include_pad=False)(x).

Key identity: pooling is linear and per-channel, so w1 @ Pool(x) = Pool(w1 @ x).
The TensorEngine does the 128->32 channel contraction on the raw stream (fp32r,
single-pass); the pooled sums are then computed on the 32-channel result y with
the VectorEngine using separable 16-block sums:
  window_j = block_{j-1} + block_j - col(16(j-1))   (j>=1), window_0 = block_0
(31-wide stride-16 windows; only the first window is clipped: counts 16 vs 31).

v2 layout: per 16-row tile, in-DMA on sync (HWDGE); the final x*s multiply is
two fused broadcast tensor_tensor ops (rows 0:8 on Vector, rows 8:16 on GpSimd)
against a stride-0 view of s; out-DMA half0 on scalar (HWDGE), half1 on gpsimd
(SWDGE).  The s-chain for tile i-1 is issued after tile i's y-matmuls so the PE
queue never stalls on the Vector chain.  All consts ride one packed DMA first.
"""

import numpy as np
from contextlib import ExitStack

import concourse.bass as bass
import concourse.tile as tile
from concourse import bacc, mybir
from concourse.bass_utils import run_bass_kernel_spmd

F32 = mybir.dt.float32
F32R = mybir.dt.float32r
AF = mybir.ActivationFunctionType

C, CR = 128, 32          # channels in / squeezed
H, W = 256, 256
NT = 16                  # h-tiles of 16 rows
TH = 16                  # rows per tile
NJ = 16                  # pooled cols
G0, G1 = 1.0 / 16.0, 1.0 / 31.0  # 1/count for edge/interior windows
NCOL = 210               # packed const columns


def _se_body(ctx, tc, out, x, cpack):
    nc = tc.nc

    consts = ctx.enter_context(tc.tile_pool(name="consts", bufs=1))
    xpool = ctx.enter_context(tc.tile_pool(name="xpool", bufs=6))
    opool = ctx.enter_context(tc.tile_pool(name="opool", bufs=5))
    small = ctx.enter_context(tc.tile_pool(name="small", bufs=3))
    ypsum = ctx.enter_context(tc.tile_pool(name="ypsum", bufs=3, space="PSUM"))
    spsum = ctx.enter_context(tc.tile_pool(name="spsum", bufs=2, space="PSUM"))

    # one packed const DMA on the sync queue ahead of the x stream
    cp = consts.tile([C, NCOL], F32)
    nc.sync.dma_start(out=cp, in_=cpack)
    w1t_s = cp[:, 0:32]            # [128, 32] lhsT of the 128->32 contraction
    qmat_s = cp[:, 32:64]          # [128, 32] 4-stacked I_32 (sums row groups)
    b2_s = cp[:, 64:65]            # [128, 1]
    b1_s = cp[0:CR, 65:66]         # [32, 1]
    w2t_s = cp[0:CR, 66:194]       # [32, 128] lhsT of the 32->128 expansion
    nj_s = cp[0:CR, 194:210]       # [32, 16] per-j 1/count_w factors

    def load(i):
        xt = xpool.tile([C, TH, NJ, 16], F32)
        if i == 0:
            # split the pipeline-fill tile so its first matmuls start ~4x
            # sooner (region-level deps let each row-quarter gate separately)
            for s in range(4):
                nc.sync.dma_start(out=xt[:, 4 * s : 4 * s + 4], in_=x[:, 4 * s : 4 * s + 4])
        else:
            nc.sync.dma_start(out=xt, in_=x[:, TH * i : TH * (i + 1)])
        # y[32q+o, hl, j, wi] = sum_c w1[o,c] * x[c, 4q+hl, 16j+wi]  (fp32r)
        y = ypsum.tile([C, 4, NJ, 16], F32)
        for q in range(4):
            for a in range(2):
                r = 4 * q + 2 * a
                nc.tensor.matmul(
                    out=y[32 * q : 32 * q + 32, 2 * a : 2 * a + 2, :, :],
                    lhsT=w1t_s,
                    rhs=xt[:, r : r + 2],
                    start=True,
                    stop=True,
                    tile_position=(0, 32 * q),
                )
        return xt, y

    tail = {"prev": None}

    def chain(i, xt, y):
        # 16-wide block sums along w, then stride-16 window sums (kernel 31)
        bs = small.tile([C, 4, NJ], F32)
        nc.vector.reduce_sum(out=bs, in_=y, axis=mybir.AxisListType.X)
        ws = small.tile([C, 4, NJ], F32)
        nc.vector.tensor_copy(out=ws[:, :, 0:1], in_=bs[:, :, 0:1])
        nc.vector.tensor_add(out=ws[:, :, 1:NJ], in0=bs[:, :, 0 : NJ - 1], in1=bs[:, :, 1:NJ])
        nc.vector.tensor_sub(out=ws[:, :, 1:NJ], in0=ws[:, :, 1:NJ], in1=y[:, :, 0 : NJ - 1, 0])

        # sum the 4 local rows per partition group, then the 4 groups via PE
        cs = small.tile([C, NJ], F32)
        nc.vector.reduce_sum(out=cs, in_=ws.rearrange("p a b -> p b a"), axis=mybir.AxisListType.X)
        full_ps = spsum.tile([CR, NJ], F32, tag="sp")
        nc.tensor.matmul(out=full_ps, lhsT=qmat_s, rhs=cs, start=True, stop=True)

        # window rows i: last 15 rows of tile i-1 + all 16 of tile i
        p_un = small.tile([CR, NJ], F32)
        if i == 0:
            nc.vector.tensor_copy(out=p_un, in_=full_ps)
        else:
            nc.vector.tensor_add(out=p_un, in0=full_ps, in1=tail["prev"])
        if i < NT - 1:
            tail_cur = small.tile([CR, NJ], F32)
            nc.vector.tensor_sub(out=tail_cur, in0=full_ps, in1=ws[0:CR, 0, :])
            tail["prev"] = tail_cur

        # h = relu(g_i * (p_un * g_j) + b1);  s = sigmoid(w2 @ h + b2)
        tmp = small.tile([CR, NJ], F32)
        nc.vector.tensor_mul(out=tmp, in0=p_un, in1=nj_s)
        h_s = small.tile([CR, NJ], F32)
        nc.scalar.activation(
            out=h_s, in_=tmp, func=AF.Relu, bias=b1_s, scale=(G0 if i == 0 else G1)
        )
        s_ps = spsum.tile([C, NJ], F32, tag="sp")
        nc.tensor.matmul(out=s_ps, lhsT=w2t_s, rhs=h_s, start=True, stop=True)
        s_s = small.tile([C, NJ], F32)
        nc.scalar.activation(out=s_s, in_=s_ps, func=AF.Sigmoid, bias=b2_s, scale=1.0)

        # out tile = x tile * s: materialize srow so the muls' innermost dim
        # has a real stride (a stride-0 inner dim runs ~3x slower on DVE),
        # then one fused multiply per row-half (Vector 0:8, GpSimd 8:16);
        # srow is built on the Scalar engine right behind the sigmoid
        srow = small.tile([C, NJ, 16], F32)
        nc.vector.tensor_copy(out=srow, in_=s_s.unsqueeze(2).broadcast_to([C, NJ, 16]))
        # 10/6 row split: DVE runs ~1.1ns/el on this mul but GpSimd ~3.2ns/el,
        # so balance per-tile busy time (DVE also carries the window chain)
        ot = opool.tile([C, TH, NJ, 16], F32)
        s_b0 = srow.unsqueeze(1).broadcast_to([C, 10, NJ, 16])
        s_b1 = srow.unsqueeze(1).broadcast_to([C, 6, NJ, 16])
        nc.vector.tensor_mul(out=ot[:, 0:10], in0=xt[:, 0:10], in1=s_b0)
        nc.gpsimd.tensor_mul(out=ot[:, 10:16], in0=xt[:, 10:16], in1=s_b1)
        nc.scalar.dma_start(out=out[:, TH * i : TH * i + 10], in_=ot[:, 0:10])
        nc.gpsimd.dma_start(out=out[:, TH * i + 10 : TH * (i + 1)], in_=ot[:, 10:16])

    # tile 0's chain is issued immediately (PE stalls are free during the
    # pipeline fill and this starts the out stream ~5us earlier); from tile 1
    # on, chain(i-1) is issued after tile i's y-matmuls so the PE queue never
    # stalls on the Vector chain in steady state
    first = load(0)
    chain(0, *first)
    prev = load(1)
    for i in range(2, NT + 1):
        cur = load(i) if i < NT else None
        chain(i - 1, *prev)
        prev = cur


def build_nc():
    nc = bacc.Bacc("TRN2", target_bir_lowering=False, debug=False)
    x = nc.dram_tensor("x", [C, H, NJ, 16], F32, kind="ExternalInput").ap()
    cpack = nc.dram_tensor("cpack", [C, NCOL], F32, kind="ExternalInput").ap()
    out = nc.dram_tensor("out", [C, H, NJ, 16], F32, kind="ExternalOutput").ap()
    with tile.TileContext(nc) as tc:
        with ExitStack() as ctx:
            _se_body(ctx, tc, out, x, cpack)
    nc.compile()
    return nc


def make_in_maps(x, w1, b1, w2, b2):
    cpack = np.zeros((C, NCOL), dtype=np.float32)
    cpack[:, 0:32] = w1.T                                    # [128, 32]
    cpack[:, 32:64] = np.tile(np.eye(CR, dtype=np.float32), (4, 1))
    cpack[:, 64] = b2
    cpack[0:CR, 65] = b1
    cpack[0:CR, 66:194] = w2.T                               # [32, 128]
    gj = np.full(NJ, G1, dtype=np.float32)
    gj[0] = G0
    cpack[0:CR, 194:210] = gj[None, :]
    return [
        {
            "x": np.ascontiguousarray(x[b]).reshape(C, H, NJ, 16),
            "cpack": cpack,
        }
        for b in range(x.shape[0])
    ]


_NC_CACHE = {}


def _get_nc():
    if "nc" not in _NC_CACHE:
        _NC_CACHE["nc"] = build_nc()
    return _NC_CACHE["nc"]


def kernel(x, w1, b1, w2, b2):
    nc = _get_nc()
    in_maps = make_in_maps(x, w1, b1, w2, b2)
    res = run_bass_kernel_spmd(nc, in_maps, core_ids=list(range(8)))
    return np.stack(
        [res.results[i]["out"].reshape(C, H, W) for i in range(8)], axis=0
    )
